# revision 27
# baseline (speedup 1.0000x reference)
"""Trainium2 Bass kernel for nn_LocalAttention (5x5 local window attention).

Contract: kernel(**inputs) takes the FULL inputs from setup_inputs() and
returns the FULL output.  Internally shards across 8 NeuronCores as
(batch b in 0..3) x (head-group hg in 0..1, 4 heads each).  Each core
computes a partial output projection; the host sums the two partials per
batch (and adds the bias half from each core).

Per-core algorithm (validated against the reference in numpy):
  - qT,kT (d-major, fp16) and v (pixel-major, fp16, with ones column for
    the softmax denominator) via matmuls from host-pre-transposed
    x.T and w slices.  PSUM->SBUF copies split between ACT and DVE.
  - k/v live in buffers padded with 2 zero image-rows top+bottom
    (buffer pixel = image pixel + 128): padded neighbors naturally give
    dots=0 -> exp(0)=1 in the denominator and v=0, matching the
    reference's zero-padded local window.
  - Per 256-pixel batch s: banded transposed pairwise dots
    E_T[j, p] = k_buf[s+j] . q[s+p] for j in [0,512) as 4 chunks of 128,
    restricted to the valid column window per chunk ([0,130) for c=0,
    [126,256) for c=3).
  - exp on ACT (scale=1/8) over the valid column slices only; in-place
    multiply by a precomputed window/wrap mask (DVE for the small c=0/3
    slices, Pool for c=1/2); column-wrapped neighbors are masked out and
    re-added to the denominator via n_pad.
  - Weighted sum over v: one accumulating matmul chain per (head,
    pixel-half) with per-chunk valid column restriction (so no memsets
    of garbage columns are needed), plus a parallel 1-column chain with
    the ones column for the denominator (pden).
  - den = pden + n_pad, reciprocal on DVE, single fused normalize
    multiply (PSUM po * broadcast recip -> fp16 opix).
  - O^T via 4 DMA-engine transposes (SP queue) -> fp16 otb; partial
    out-projection on PE; bias add (0.5*b_out) on Pool -> fp16 out DMA.
"""

import numpy as np

B, HMAP, WMAP = 4, 64, 64
N = HMAP * WMAP          # 4096
DIM = 512
HEADS, HEAD_DIM = 8, 64
INNER = HEADS * HEAD_DIM  # 512
SCALE = HEAD_DIM ** -0.5
NB = N + 256             # padded k/v buffer pixels (2 zero rows each side)
NCHUNK = NB // 128       # 34
N_CORES = 8

_cache = {}


def _make_masks():
    """Window/wrap masks for the 4 chunks of a 256-px batch, plus n_pad.

    mask[c, j', p'] = 1 iff o = 128*c + j' - p' - 128 decomposes as
    64*di + dj with |di|,|dj| <= 2 and column p'%64 + dj stays in-image.
    n_pad[p] = number of column-invalid window positions for column p%64.
    """
    o = (128 * np.arange(4)[:, None, None] + np.arange(128)[None, :, None]
         - np.arange(256)[None, None, :] - 128)           # [4,128,256]
    di = np.round(o / 64.0).astype(np.int64)
    dj = o - 64 * di
    col = (np.arange(256) % 64)[None, None, :]
    ok = (np.abs(di) <= 2) & (np.abs(dj) <= 2) & (col + dj >= 0) & (col + dj < 64)
    masks = ok.astype(np.float16)
    colv = np.arange(64)
    npad_col = np.zeros(64, dtype=np.float32)
    for djv in range(-2, 3):
        npad_col += 5.0 * ((colv + djv < 0) | (colv + djv >= 64))
    n_pad = np.tile(npad_col, 2).reshape(128, 1).astype(np.float32)
    return masks, n_pad


# valid q-column window [lo, hi) of each 128-row k-chunk of a 256-px batch
CHUNK_WIN = [(0, 130), (0, 256), (0, 256), (126, 256)]


def _build_nc(stage=99):
    import os
    stage = int(os.environ.get("KSTAGE", stage))
    import concourse.bass as bass
    import concourse.tile as tile
    from concourse import mybir

    f32 = mybir.dt.float32
    f16 = mybir.dt.float16
    Exp = mybir.ActivationFunctionType.Exp

    from concourse import bacc
    nc = bacc.Bacc(None, target_bir_lowering=False)
    xt_d = nc.dram_tensor("xt", [DIM, N], f16, kind="ExternalInput")
    wqkvt_d = nc.dram_tensor("wqkvt", [DIM, 768], f16, kind="ExternalInput")
    woutt_d = nc.dram_tensor("woutt", [256, DIM], f16, kind="ExternalInput")
    masks_d = nc.dram_tensor("masks", [4, 128, 256], f16, kind="ExternalInput")
    npad_d = nc.dram_tensor("npad", [128, 1], f32, kind="ExternalInput")
    ident_d = nc.dram_tensor("ident", [128, 128], f16, kind="ExternalInput")
    out_d = nc.dram_tensor("out", [N, DIM], f16, kind="ExternalOutput")

    with tile.TileContext(nc) as tc:
        from contextlib import ExitStack
        with ExitStack() as ctx:
            consts = ctx.enter_context(tc.tile_pool(name="consts", bufs=1))

            wqkvt = consts.tile([128, 4, 768], f16)
            nc.sync.dma_start(out=wqkvt,
                              in_=wqkvt_d.rearrange("(c p) m -> p c m", p=128))
            woutt = consts.tile([128, 2, DIM], f16)
            nc.sync.dma_start(out=woutt,
                              in_=woutt_d.rearrange("(c p) m -> p c m", p=128))
            masks = consts.tile([128, 4, 256], f16)
            nc.sync.dma_start(out=masks,
                              in_=masks_d.rearrange("c p f -> p c f"))
            npad = consts.tile([128, 1], f32)
            nc.sync.dma_start(out=npad, in_=npad_d[:, :])
            ident = consts.tile([128, 128], f16)
            nc.sync.dma_start(out=ident, in_=ident_d[:, :])

            # persistent activations: [p, head-pair g, pixel]
            qt = consts.tile([128, 2, N], f16)
            kt = consts.tile([128, 2, NB], f16)
            # v buffer: [p, chunk, 4 heads x (64 + ones col)]
            vsb = consts.tile([128, NCHUNK, 260], f16)

            nc.vector.memset(kt[:, :, 0:128], 0.0)
            nc.vector.memset(kt[:, :, NB - 128:NB], 0.0)
            nc.vector.memset(vsb[:, 0, :], 0.0)
            nc.vector.memset(vsb[:, NCHUNK - 1, :], 0.0)
            # ones columns (after zero memsets of the pad chunks)
            ones_ap = vsb.rearrange("p c (h e) -> p c h e", h=4)[:, :, :, 64:65]
            nc.vector.memset(ones_ap, 1.0)

            # ---------------- fused pipeline ----------------
            with ExitStack() as cctx:
              if stage >= 2:
                  pspw = cctx.enter_context(
                      tc.tile_pool(name="psum_pw", bufs=2, space="PSUM"))
                  psb = cctx.enter_context(
                      tc.tile_pool(name="psum_b", bufs=1, space="PSUM"))
                  pso = cctx.enter_context(
                      tc.tile_pool(name="psum_o", bufs=1, space="PSUM"))
                  psd = cctx.enter_context(
                      tc.tile_pool(name="psum_d", bufs=1, space="PSUM"))
                  # pj (out-projection) and pt (transposes) share one bank:
                  # their lifetimes are staggered within an iteration.
                  pjt = cctx.enter_context(
                      tc.tile_pool(name="psum_pjt", bufs=1, space="PSUM"))
                  xin = cctx.enter_context(tc.tile_pool(name="xin", bufs=2))
                  epool = cctx.enter_context(tc.tile_pool(name="em", bufs=3))
                  dpool = cctx.enter_context(tc.tile_pool(name="den", bufs=2))
                  opool = cctx.enter_context(tc.tile_pool(name="opix", bufs=2))
                  otpool = cctx.enter_context(tc.tile_pool(name="ot", bufs=2))
                  obpool = cctx.enter_context(tc.tile_pool(name="ob", bufs=2))
                  xt_view = xt_d.rearrange("(c p) n -> p c n", p=128)

                  def strip_proj(t):
                      """q/k/v projection for the 256-px strip [256t, 256t+256).

                      Emitted two iterations ahead of first use; the matmuls
                      are PE filler work behind the critical dots/AV stream.
                      """
                      s0 = 256 * t
                      xtile = xin.tile([128, 4, 256], f16)
                      nc.sync.dma_start(out=xtile, in_=xt_view[:, :, s0:s0 + 256])
                      for qk in range(2):  # 0: q pair0/1, 1: k pair0/1
                          ps = psb.tile([128, 2, 256], f32, tag="ps", name="psqk")
                          for m in range(2):
                              for kc in range(4):
                                  nc.tensor.matmul(
                                      ps[:, m, :],
                                      wqkvt[:, kc, (2 * qk + m) * 128:
                                            (2 * qk + m + 1) * 128],
                                      xtile[:, kc, :],
                                      start=(kc == 0), stop=(kc == 3))
                          if qk == 0:
                              nc.scalar.copy(qt[:, :, s0:s0 + 256], ps)
                          else:
                              nc.vector.tensor_copy(
                                  kt[:, :, 128 + s0:128 + s0 + 256], ps)
                      psv = psb.tile([128, 2, 256], f32, tag="ps", name="psv")
                      for sub in range(2):
                          for kc in range(4):
                              nc.tensor.matmul(
                                  psv[:, sub, :],
                                  xtile[:, kc, sub * 128:(sub + 1) * 128],
                                  wqkvt[:, kc, 512:768],
                                  start=(kc == 0), stop=(kc == 3))
                      vdst = vsb[:, 1 + 2 * t:3 + 2 * t].rearrange(
                          "p c (h e) -> p c h e", h=4)[:, :, :, 0:64]
                      nc.vector.tensor_copy(
                          vdst, psv.rearrange("p c (h e) -> p c h e", h=4))

                  def dots_exp_mask(si):
                      """QK^T dots + exp + mask for batch si -> em tile."""
                      s = si * 256
                      # em[j', chunk, slot, p']; slot = 2*(h%2) + h//2
                      em = epool.tile([128, 4, 4, 256], f16)
                      if si < 3:
                          # zero the never-exp'd zones once per buffer; the
                          # si loop only ever writes inside CHUNK_WIN after.
                          nc.vector.memset(em[:, 0, :, 130:256], 0.0)
                          nc.vector.memset(em[:, 3, :, 0:126], 0.0)
                      for c in range(4):
                          lo, hi = CHUNK_WIN[c]
                          pw = pspw.tile([128, 2, 2, 256], f32)
                          for g in range(2):
                              for hs in range(2):
                                  lo_p, hi_p = hs * 64, (hs + 1) * 64
                                  nc.tensor.matmul(
                                      pw[:, hs, g, lo:hi],
                                      kt[lo_p:hi_p, g,
                                         s + 128 * c:s + 128 * c + 128],
                                      qt[lo_p:hi_p, g, s + lo:s + hi],
                                      start=True, stop=True)
                          nc.scalar.activation(
                              out=em[:, c, :, lo:hi],
                              in_=pw.rearrange("p a b f -> p (a b) f")[:, :, lo:hi],
                              func=Exp, scale=SCALE)
                          mask_b = masks[:, c, lo:hi].unsqueeze(1).to_broadcast(
                              [128, 4, hi - lo])
                          nc.vector.tensor_mul(em[:, c, :, lo:hi],
                                               em[:, c, :, lo:hi], mask_b)
                      return em

                  def av(si, em):
                      """Attention-weighted V sums + denominators for batch si."""
                      # po[p', ph, gh, e]; pden[p', ph, gh, 0] via ones column
                      po = pso.tile([128, 2, 4, 64], f32)
                      pden = psd.tile([128, 2, 4, 1], f32)
                      for gh in range(4):
                          slot = 2 * (gh % 2) + gh // 2
                          for ph in range(2):
                              plo, phi = 128 * ph, 128 * ph + 128
                              for out_ap, vlo, vn in (
                                      (po, 65 * gh, 64),
                                      (pden, 65 * gh + 64, 1)):
                                  for c in range(4):
                                      nc.tensor.matmul(
                                          out_ap[:, ph, gh, :],
                                          em[:, c, slot, plo:phi],
                                          vsb[:, 2 * si + c, vlo:vlo + vn],
                                          start=(c == 0), stop=(c == 3))
                      return po, pden

                  def post(po, pden):
                      """Normalize + transpose: po/pden -> otb (O^T, fp16)."""
                      den = dpool.tile([128, 2, 4], f32, tag="den")
                      nc.vector.tensor_add(
                          den, pden[:, :, :, 0],
                          npad.unsqueeze(1).to_broadcast([128, 2, 4]))
                      rec = dpool.tile([128, 2, 4], f32, tag="rec")
                      nc.vector.reciprocal(rec, den)
                      opix = opool.tile([128, 2, 4, 64], f16)
                      nc.vector.tensor_mul(
                          opix, po, rec.unsqueeze(3).to_broadcast([128, 2, 4, 64]))
                      # O^T via 4 PE transposes (f16): pt[e-local, ph, i, p]
                      pt = pjt.tile([128, 2, 2, 128], f16, tag="pjt", name="pt")
                      for ph in range(2):
                          for i in range(2):
                              nc.tensor.transpose(
                                  pt[:, ph, i],
                                  opix[:, ph, 2 * i:2 * i + 2, :].rearrange(
                                      "p a e -> p (a e)"),
                                  ident)
                      otb = otpool.tile([128, 2, 2, 128], f16)
                      nc.vector.tensor_copy(otb, pt)
                      return otb

                  def outproj(si, otb):
                      """Partial out-projection + bias for batch si -> DMA out."""
                      s = si * 256
                      ob = obpool.tile([128, 2, DIM], f16)
                      for ph in range(2):
                          pj = pjt.tile([128, DIM], f32, tag="pjt", name="pj")
                          for i in range(2):
                              nc.tensor.matmul(
                                  pj, otb[:, ph, i], woutt[:, i],
                                  start=(i == 0), stop=(i == 1))
                          nc.scalar.copy(ob[:, ph], pj)
                      nc.sync.dma_start(
                          out=out_d[s:s + 256, :].rearrange(
                              "(a p) m -> p a m", a=2, p=128),
                          in_=ob)

                  # Software-pipelined emission: per iteration emit dots(si)
                  # first (lowest priority -> PE prefers them when ready),
                  # then the previous batch's AV, the out-projection two
                  # batches back, the previous batch's normalize/transpose,
                  # and finally the strip projection two batches ahead as PE
                  # filler.  This matches true readiness order so the
                  # scheduler's in-order PE stream never blocks on a
                  # cross-engine chain while newer work is ready.
                  ems, otbs = {}, {}
                  strip_proj(0)
                  strip_proj(1)
                  for si in range(18):
                      if si < 16:
                          ems[si] = dots_exp_mask(si)
                      if stage >= 3 and 1 <= si <= 16:
                          po, pden = av(si - 1, ems.pop(si - 1))
                      if stage >= 4 and si >= 2 and (si - 2) in otbs:
                          outproj(si - 2, otbs.pop(si - 2))
                      if stage >= 4 and 1 <= si <= 16:
                          otbs[si - 1] = post(po, pden)
                      if si + 2 <= 15:
                          strip_proj(si + 2)

    nc.finalize()
    return nc


def _prepare_core_inputs(x, w_qkv, w_out, b_out):
    masks, n_pad = _make_masks()
    ident = np.eye(128, dtype=np.float16)
    per_core = []
    for ci in range(N_CORES):
        b, hg = ci // 2, ci % 2
        q_rows = w_qkv[256 * hg:256 * hg + 256]
        k_rows = w_qkv[INNER + 256 * hg:INNER + 256 * hg + 256]
        v_rows = w_qkv[2 * INNER + 256 * hg:2 * INNER + 256 * hg + 256]
        w_slice = np.concatenate([q_rows, k_rows, v_rows], axis=0)  # [768, 512]
        per_core.append({
            "xt": np.ascontiguousarray(x[b].T).astype(np.float16),
            "wqkvt": np.ascontiguousarray(w_slice.T).astype(np.float16),
            "woutt": np.ascontiguousarray(
                w_out[:, 256 * hg:256 * hg + 256].T).astype(np.float16),
            "masks": masks,
            "npad": n_pad,
            "ident": ident,
        })
    return per_core


def kernel(x, w_qkv, w_out, b_out, h, w):
    assert int(h) == HMAP and int(w) == WMAP
    x = np.asarray(x, dtype=np.float32)
    w_qkv = np.asarray(w_qkv, dtype=np.float32)
    w_out = np.asarray(w_out, dtype=np.float32)
    b_out = np.asarray(b_out, dtype=np.float32)

    if "nc" not in _cache:
        _cache["nc"] = _build_nc()
    nc = _cache["nc"]

    from concourse.bass_utils import run_bass_kernel_spmd
    in_maps = _prepare_core_inputs(x, w_qkv, w_out, b_out)
    res = run_bass_kernel_spmd(nc, in_maps, core_ids=list(range(N_CORES)))
    out = np.zeros((B, N, DIM), dtype=np.float32)
    for b in range(B):
        out[b] = (res.results[2 * b]["out"].astype(np.float32)
                  + res.results[2 * b + 1]["out"].astype(np.float32)
                  + b_out)
    return out


# revision 32
# speedup vs baseline: 1.0145x; 1.0145x over previous
"""Trainium2 Bass kernel for nn_LocalAttention (5x5 local window attention).

Contract: kernel(**inputs) takes the FULL inputs from setup_inputs() and
returns the FULL output.  Internally shards across 8 NeuronCores as
(batch b in 0..3) x (head-group hg in 0..1, 4 heads each).  Each core
computes a partial output projection; the host sums the two partials per
batch (and adds the bias half from each core).

Single software-pipelined loop (no separate projection phase): each
iteration emits, in priority order, the QK^T dots/exp/mask for batch si,
the attention-weighted V sums for batch si-1, the out-projection for
batch si-2, the normalize/transpose for batch si-1, and the q/k/v
projection for the 256-px strip si+2 (PE filler work).  This keeps the
PE busy ~77% of the makespan and ramped at full clock.

Per-core algorithm (validated against the reference in numpy):
  - qT,kT (d-major, fp16) and v (pixel-major, fp16, with ones column for
    the softmax denominator) via matmuls from host-pre-transposed
    x.T and w slices, one 256-px strip per iteration.
  - k/v live in buffers padded with 2 zero image-rows top+bottom
    (buffer pixel = image pixel + 128): padded neighbors naturally give
    dots=0 -> exp(0)=1 in the denominator and v=0, matching the
    reference's zero-padded local window.
  - Per 256-pixel batch s: banded transposed pairwise dots
    E_T[j, p] = k_buf[s+j] . q[s+p] for j in [0,512) as 4 chunks of 128,
    restricted to the valid column window per chunk ([0,130) for c=0,
    [126,256) for c=3).
  - exp on ACT (scale=1/8) over the valid column slices only; in-place
    multiply by a precomputed window/wrap mask on DVE (fp16 2x mode;
    GPSIMD cannot read PSUM and is 2.6x slower per element, so it gets
    no regular work); column-wrapped neighbors are masked out and
    re-added to the denominator via n_pad.
  - Weighted sum over v: one accumulating matmul chain per (head,
    pixel-half); out-of-window em columns are zeroed once per em buffer
    so full-width stationaries are safe.  A parallel 1-column chain
    with the ones column accumulates the denominator (pden).
  - den = pden + n_pad, reciprocal on DVE, single fused normalize
    multiply (PSUM po * broadcast recip -> fp16 opix).
  - O^T via 4 fp16 PE transposes into one PSUM bank shared with the
    out-projection accumulator (staggered lifetimes); psum->sbuf copies
    split between ACT and DVE; b_out is added on the host during the
    partial-sum gather; fp16 output DMA.
"""

import numpy as np

B, HMAP, WMAP = 4, 64, 64
N = HMAP * WMAP          # 4096
DIM = 512
HEADS, HEAD_DIM = 8, 64
INNER = HEADS * HEAD_DIM  # 512
SCALE = HEAD_DIM ** -0.5
NB = N + 256             # padded k/v buffer pixels (2 zero rows each side)
NCHUNK = NB // 128       # 34
N_CORES = 8

_cache = {}


def _make_masks():
    """Window/wrap masks for the 4 chunks of a 256-px batch, plus n_pad.

    mask[c, j', p'] = 1 iff o = 128*c + j' - p' - 128 decomposes as
    64*di + dj with |di|,|dj| <= 2 and column p'%64 + dj stays in-image.
    n_pad[p] = number of column-invalid window positions for column p%64.
    """
    o = (128 * np.arange(4)[:, None, None] + np.arange(128)[None, :, None]
         - np.arange(256)[None, None, :] - 128)           # [4,128,256]
    di = np.round(o / 64.0).astype(np.int64)
    dj = o - 64 * di
    col = (np.arange(256) % 64)[None, None, :]
    ok = (np.abs(di) <= 2) & (np.abs(dj) <= 2) & (col + dj >= 0) & (col + dj < 64)
    masks = ok.astype(np.float16)
    colv = np.arange(64)
    npad_col = np.zeros(64, dtype=np.float32)
    for djv in range(-2, 3):
        npad_col += 5.0 * ((colv + djv < 0) | (colv + djv >= 64))
    n_pad = np.tile(npad_col, 2).reshape(128, 1).astype(np.float32)
    return masks, n_pad


# valid q-column window [lo, hi) of each 128-row k-chunk of a 256-px batch
CHUNK_WIN = [(0, 130), (0, 256), (0, 256), (126, 256)]


def _build_nc(stage=99):
    import os
    stage = int(os.environ.get("KSTAGE", stage))
    import concourse.bass as bass
    import concourse.tile as tile
    from concourse import mybir

    f32 = mybir.dt.float32
    f16 = mybir.dt.float16
    Exp = mybir.ActivationFunctionType.Exp

    from concourse import bacc
    nc = bacc.Bacc(None, target_bir_lowering=False)
    xt_d = nc.dram_tensor("xt", [DIM, N], f16, kind="ExternalInput")
    wqkvt_d = nc.dram_tensor("wqkvt", [DIM, 768], f16, kind="ExternalInput")
    woutt_d = nc.dram_tensor("woutt", [256, DIM], f16, kind="ExternalInput")
    masks_d = nc.dram_tensor("masks", [4, 128, 256], f16, kind="ExternalInput")
    npad_d = nc.dram_tensor("npad", [128, 1], f32, kind="ExternalInput")
    ident_d = nc.dram_tensor("ident", [128, 128], f16, kind="ExternalInput")
    out_d = nc.dram_tensor("out", [N, DIM], f16, kind="ExternalOutput")

    with tile.TileContext(nc) as tc:
        from contextlib import ExitStack
        with ExitStack() as ctx:
            consts = ctx.enter_context(tc.tile_pool(name="consts", bufs=1))

            # wqkvt first: the prologue strips need it immediately; the
            # other consts are DMA'd after the prologue (first used later).
            wqkvt = consts.tile([128, 4, 768], f16)
            wqkvt_v = wqkvt_d.rearrange("(c p) m -> p c m", p=128)
            nc.sync.dma_start(out=wqkvt[:, :, 0:512], in_=wqkvt_v[:, :, 0:512])
            nc.sync.dma_start(out=wqkvt[:, :, 512:768], in_=wqkvt_v[:, :, 512:768])
            woutt = consts.tile([128, 2, DIM], f16)
            masks = consts.tile([128, 4, 256], f16)
            npad = consts.tile([128, 1], f32)
            ident = consts.tile([128, 128], f16)

            def late_const_dmas():
                nc.sync.dma_start(out=woutt,
                                  in_=woutt_d.rearrange("(c p) m -> p c m", p=128))
                nc.sync.dma_start(out=masks,
                                  in_=masks_d.rearrange("c p f -> p c f"))
                nc.sync.dma_start(out=npad, in_=npad_d[:, :])
                nc.sync.dma_start(out=ident, in_=ident_d[:, :])

            # persistent activations: [p, head-pair g, pixel]
            qt = consts.tile([128, 2, N], f16)
            kt = consts.tile([128, 2, NB], f16)
            # v buffer: [p, chunk, 4 heads x (64 + ones col)]
            vsb = consts.tile([128, NCHUNK, 260], f16)

            nc.vector.memset(kt[:, :, 0:128], 0.0)
            nc.vector.memset(kt[:, :, NB - 128:NB], 0.0)
            nc.vector.memset(vsb[:, 0, :], 0.0)
            nc.vector.memset(vsb[:, NCHUNK - 1, :], 0.0)
            # ones columns (after zero memsets of the pad chunks)
            ones_ap = vsb.rearrange("p c (h e) -> p c h e", h=4)[:, :, :, 64:65]
            nc.vector.memset(ones_ap, 1.0)

            # ---------------- fused pipeline ----------------
            with ExitStack() as cctx:
              if stage >= 2:
                  pspw = cctx.enter_context(
                      tc.tile_pool(name="psum_pw", bufs=2, space="PSUM"))
                  psb = cctx.enter_context(
                      tc.tile_pool(name="psum_b", bufs=1, space="PSUM"))
                  pso = cctx.enter_context(
                      tc.tile_pool(name="psum_o", bufs=1, space="PSUM"))
                  psd = cctx.enter_context(
                      tc.tile_pool(name="psum_d", bufs=1, space="PSUM"))
                  # pj (out-projection) and pt (transposes) share one bank:
                  # their lifetimes are staggered within an iteration.
                  pjt = cctx.enter_context(
                      tc.tile_pool(name="psum_pjt", bufs=1, space="PSUM"))
                  xin = cctx.enter_context(tc.tile_pool(name="xin", bufs=2))
                  epool = cctx.enter_context(tc.tile_pool(name="em", bufs=3))
                  dpool = cctx.enter_context(tc.tile_pool(name="den", bufs=2))
                  opool = cctx.enter_context(tc.tile_pool(name="opix", bufs=2))
                  otpool = cctx.enter_context(tc.tile_pool(name="ot", bufs=2))
                  obpool = cctx.enter_context(tc.tile_pool(name="ob", bufs=2))
                  xt_view = xt_d.rearrange("(c p) n -> p c n", p=128)
                  pre_x = {}
                  for t in range(2):
                      xt_pre = xin.tile([128, 4, 256], f16, name=f"xt_pre{t}")
                      nc.sync.dma_start(out=xt_pre,
                                        in_=xt_view[:, :, 256 * t:256 * t + 256])
                      pre_x[t] = xt_pre

                  def strip_proj(t):
                      """q/k/v projection for the 256-px strip [256t, 256t+256).

                      Emitted two iterations ahead of first use; the matmuls
                      are PE filler work behind the critical dots/AV stream.
                      """
                      s0 = 256 * t
                      if t in pre_x:
                          xtile = pre_x.pop(t)
                      else:
                          xtile = xin.tile([128, 4, 256], f16)
                          nc.sync.dma_start(out=xtile,
                                            in_=xt_view[:, :, s0:s0 + 256])
                      for qk in range(2):  # 0: q pair0/1, 1: k pair0/1
                          ps = psb.tile([128, 2, 256], f32, tag="ps", name="psqk")
                          for m in range(2):
                              for kc in range(4):
                                  nc.tensor.matmul(
                                      ps[:, m, :],
                                      wqkvt[:, kc, (2 * qk + m) * 128:
                                            (2 * qk + m + 1) * 128],
                                      xtile[:, kc, :],
                                      start=(kc == 0), stop=(kc == 3))
                          if qk == 0:
                              nc.scalar.copy(qt[:, :, s0:s0 + 256], ps)
                          else:
                              nc.vector.tensor_copy(
                                  kt[:, :, 128 + s0:128 + s0 + 256], ps)
                      psv = psb.tile([128, 2, 256], f32, tag="ps", name="psv")
                      for sub in range(2):
                          for kc in range(4):
                              nc.tensor.matmul(
                                  psv[:, sub, :],
                                  xtile[:, kc, sub * 128:(sub + 1) * 128],
                                  wqkvt[:, kc, 512:768],
                                  start=(kc == 0), stop=(kc == 3))
                      vdst = vsb[:, 1 + 2 * t:3 + 2 * t].rearrange(
                          "p c (h e) -> p c h e", h=4)[:, :, :, 0:64]
                      nc.vector.tensor_copy(
                          vdst, psv.rearrange("p c (h e) -> p c h e", h=4))

                  def dots_exp_mask(si):
                      """QK^T dots + exp + mask for batch si -> em tile."""
                      s = si * 256
                      # em[j', chunk, slot, p']; slot = 2*(h%2) + h//2
                      em = epool.tile([128, 4, 4, 256], f16)
                      if si < 3:
                          # zero the never-exp'd zones once per buffer; the
                          # si loop only ever writes inside CHUNK_WIN after.
                          nc.vector.memset(em[:, 0, :, 130:256], 0.0)
                          nc.vector.memset(em[:, 3, :, 0:126], 0.0)
                      for c in range(4):
                          lo, hi = CHUNK_WIN[c]
                          pw = pspw.tile([128, 2, 2, 256], f32)
                          for g in range(2):
                              for hs in range(2):
                                  lo_p, hi_p = hs * 64, (hs + 1) * 64
                                  nc.tensor.matmul(
                                      pw[:, hs, g, lo:hi],
                                      kt[lo_p:hi_p, g,
                                         s + 128 * c:s + 128 * c + 128],
                                      qt[lo_p:hi_p, g, s + lo:s + hi],
                                      start=True, stop=True)
                          nc.scalar.activation(
                              out=em[:, c, :, lo:hi],
                              in_=pw.rearrange("p a b f -> p (a b) f")[:, :, lo:hi],
                              func=Exp, scale=SCALE)
                          mask_b = masks[:, c, lo:hi].unsqueeze(1).to_broadcast(
                              [128, 4, hi - lo])
                          nc.vector.tensor_mul(em[:, c, :, lo:hi],
                                               em[:, c, :, lo:hi], mask_b)
                      return em

                  def av(si, em):
                      """Attention-weighted V sums + denominators for batch si."""
                      # po[p', ph, gh, e]; pden[p', ph, gh, 0] via ones column
                      po = pso.tile([128, 2, 4, 64], f32)
                      pden = psd.tile([128, 2, 4, 1], f32)
                      for gh in range(4):
                          slot = 2 * (gh % 2) + gh // 2
                          for ph in range(2):
                              plo, phi = 128 * ph, 128 * ph + 128
                              for out_ap, vlo, vn in (
                                      (po, 65 * gh, 64),
                                      (pden, 65 * gh + 64, 1)):
                                  for c in range(4):
                                      nc.tensor.matmul(
                                          out_ap[:, ph, gh, :],
                                          em[:, c, slot, plo:phi],
                                          vsb[:, 2 * si + c, vlo:vlo + vn],
                                          start=(c == 0), stop=(c == 3))
                      return po, pden

                  def post(po, pden):
                      """Normalize + transpose: po/pden -> otb (O^T, fp16)."""
                      den = dpool.tile([128, 2, 4], f32, tag="den")
                      nc.vector.tensor_add(
                          den, pden[:, :, :, 0],
                          npad.unsqueeze(1).to_broadcast([128, 2, 4]))
                      rec = dpool.tile([128, 2, 4], f32, tag="rec")
                      nc.vector.reciprocal(rec, den)
                      opix = opool.tile([128, 2, 4, 64], f16)
                      nc.vector.tensor_mul(
                          opix, po, rec.unsqueeze(3).to_broadcast([128, 2, 4, 64]))
                      # O^T via 4 PE transposes (f16): pt[e-local, ph, i, p]
                      pt = pjt.tile([128, 2, 2, 128], f16, tag="pjt", name="pt")
                      for ph in range(2):
                          for i in range(2):
                              nc.tensor.transpose(
                                  pt[:, ph, i],
                                  opix[:, ph, 2 * i:2 * i + 2, :].rearrange(
                                      "p a e -> p (a e)"),
                                  ident)
                      otb = otpool.tile([128, 2, 2, 128], f16)
                      nc.vector.tensor_copy(otb, pt)
                      return otb

                  def outproj(si, otb):
                      """Partial out-projection + bias for batch si -> DMA out."""
                      s = si * 256
                      ob = obpool.tile([128, 2, DIM], f16)
                      for ph in range(2):
                          pj = pjt.tile([128, DIM], f32, tag="pjt", name="pj")
                          for i in range(2):
                              nc.tensor.matmul(
                                  pj, otb[:, ph, i], woutt[:, i],
                                  start=(i == 0), stop=(i == 1))
                          nc.scalar.copy(ob[:, ph], pj)
                      nc.sync.dma_start(
                          out=out_d[s:s + 256, :].rearrange(
                              "(a p) m -> p a m", a=2, p=128),
                          in_=ob)

                  # Software-pipelined emission: per iteration emit dots(si)
                  # first (lowest priority -> PE prefers them when ready),
                  # then the previous batch's AV, the out-projection two
                  # batches back, the previous batch's normalize/transpose,
                  # and finally the strip projection two batches ahead as PE
                  # filler.  This matches true readiness order so the
                  # scheduler's in-order PE stream never blocks on a
                  # cross-engine chain while newer work is ready.
                  ems, otbs = {}, {}
                  strip_proj(0)
                  strip_proj(1)
                  late_const_dmas()
                  for si in range(18):
                      if si < 16:
                          ems[si] = dots_exp_mask(si)
                      if stage >= 3 and 1 <= si <= 16:
                          po, pden = av(si - 1, ems.pop(si - 1))
                      if stage >= 4 and si >= 2 and (si - 2) in otbs:
                          outproj(si - 2, otbs.pop(si - 2))
                      if stage >= 4 and 1 <= si <= 16:
                          otbs[si - 1] = post(po, pden)
                      if si + 2 <= 15:
                          strip_proj(si + 2)

    nc.finalize()
    return nc


def _prepare_core_inputs(x, w_qkv, w_out, b_out):
    masks, n_pad = _make_masks()
    ident = np.eye(128, dtype=np.float16)
    per_core = []
    for ci in range(N_CORES):
        b, hg = ci // 2, ci % 2
        q_rows = w_qkv[256 * hg:256 * hg + 256]
        k_rows = w_qkv[INNER + 256 * hg:INNER + 256 * hg + 256]
        v_rows = w_qkv[2 * INNER + 256 * hg:2 * INNER + 256 * hg + 256]
        w_slice = np.concatenate([q_rows, k_rows, v_rows], axis=0)  # [768, 512]
        per_core.append({
            "xt": np.ascontiguousarray(x[b].T).astype(np.float16),
            "wqkvt": np.ascontiguousarray(w_slice.T).astype(np.float16),
            "woutt": np.ascontiguousarray(
                w_out[:, 256 * hg:256 * hg + 256].T).astype(np.float16),
            "masks": masks,
            "npad": n_pad,
            "ident": ident,
        })
    return per_core


def kernel(x, w_qkv, w_out, b_out, h, w):
    assert int(h) == HMAP and int(w) == WMAP
    x = np.asarray(x, dtype=np.float32)
    w_qkv = np.asarray(w_qkv, dtype=np.float32)
    w_out = np.asarray(w_out, dtype=np.float32)
    b_out = np.asarray(b_out, dtype=np.float32)

    if "nc" not in _cache:
        _cache["nc"] = _build_nc()
    nc = _cache["nc"]

    from concourse.bass_utils import run_bass_kernel_spmd
    in_maps = _prepare_core_inputs(x, w_qkv, w_out, b_out)
    res = run_bass_kernel_spmd(nc, in_maps, core_ids=list(range(N_CORES)))
    out = np.zeros((B, N, DIM), dtype=np.float32)
    for b in range(B):
        out[b] = (res.results[2 * b]["out"].astype(np.float32)
                  + res.results[2 * b + 1]["out"].astype(np.float32)
                  + b_out)
    return out


# revision 34
# speedup vs baseline: 1.0244x; 1.0097x over previous
"""Trainium2 Bass kernel for nn_LocalAttention (5x5 local window attention).

Contract: kernel(**inputs) takes the FULL inputs from setup_inputs() and
returns the FULL output.  Internally shards across 8 NeuronCores as
(batch b in 0..3) x (head-group hg in 0..1, 4 heads each).  Each core
computes a partial output projection; the host sums the two partials per
batch (and adds the bias half from each core).

Single software-pipelined loop (no separate projection phase): each
iteration emits, in priority order, the QK^T dots/exp/mask for batch si,
the attention-weighted V sums for batch si-1, the out-projection for
batch si-2, the normalize/transpose for batch si-1, and the q/k/v
projection for the 256-px strip si+2 (PE filler work).  This keeps the
PE busy ~77% of the makespan and ramped at full clock.

Per-core algorithm (validated against the reference in numpy):
  - qT,kT (d-major, fp16) and v (pixel-major, fp16, with ones column for
    the softmax denominator) via matmuls from host-pre-transposed
    x.T and w slices, one 256-px strip per iteration.
  - k/v live in buffers padded with 2 zero image-rows top+bottom
    (buffer pixel = image pixel + 128): padded neighbors naturally give
    dots=0 -> exp(0)=1 in the denominator and v=0, matching the
    reference's zero-padded local window.
  - Per 256-pixel batch s: banded transposed pairwise dots
    E_T[j, p] = k_buf[s+j] . q[s+p] for j in [0,512) as 4 chunks of 128,
    restricted to the valid column window per chunk ([0,130) for c=0,
    [126,256) for c=3).
  - exp on ACT (scale=1/8) over the valid column slices only; in-place
    multiply by a precomputed window/wrap mask on DVE (fp16 2x mode;
    GPSIMD cannot read PSUM and is 2.6x slower per element, so it gets
    no regular work); column-wrapped neighbors are masked out and
    re-added to the denominator via n_pad.
  - Weighted sum over v: one accumulating matmul chain per (head,
    pixel-half); out-of-window em columns are zeroed once per em buffer
    so full-width stationaries are safe.  A parallel 1-column chain
    with the ones column accumulates the denominator (pden).
  - den = pden + n_pad, reciprocal on DVE, single fused normalize
    multiply (PSUM po * broadcast recip -> fp16 opix).
  - O^T via 4 fp16 PE transposes into one PSUM bank shared with the
    out-projection accumulator (staggered lifetimes); psum->sbuf copies
    split between ACT and DVE; b_out is added on the host during the
    partial-sum gather; fp16 output DMA.
"""

import numpy as np

B, HMAP, WMAP = 4, 64, 64
N = HMAP * WMAP          # 4096
DIM = 512
HEADS, HEAD_DIM = 8, 64
INNER = HEADS * HEAD_DIM  # 512
SCALE = HEAD_DIM ** -0.5
NB = N + 256             # padded k/v buffer pixels (2 zero rows each side)
NCHUNK = NB // 128       # 34
N_CORES = 8

_cache = {}


def _make_masks():
    """Window/wrap masks for the 4 chunks of a 256-px batch, plus n_pad.

    mask[c, j', p'] = 1 iff o = 128*c + j' - p' - 128 decomposes as
    64*di + dj with |di|,|dj| <= 2 and column p'%64 + dj stays in-image.
    n_pad[p] = number of column-invalid window positions for column p%64.
    """
    o = (128 * np.arange(4)[:, None, None] + np.arange(128)[None, :, None]
         - np.arange(256)[None, None, :] - 128)           # [4,128,256]
    di = np.round(o / 64.0).astype(np.int64)
    dj = o - 64 * di
    col = (np.arange(256) % 64)[None, None, :]
    ok = (np.abs(di) <= 2) & (np.abs(dj) <= 2) & (col + dj >= 0) & (col + dj < 64)
    masks = ok.astype(np.float16)
    colv = np.arange(64)
    npad_col = np.zeros(64, dtype=np.float32)
    for djv in range(-2, 3):
        npad_col += 5.0 * ((colv + djv < 0) | (colv + djv >= 64))
    n_pad = np.tile(npad_col, 2).reshape(128, 1).astype(np.float32)
    return masks, n_pad


# valid q-column window [lo, hi) of each 128-row k-chunk of a 256-px batch
CHUNK_WIN = [(0, 130), (0, 256), (0, 256), (126, 256)]


def _build_nc(stage=99):
    import os
    stage = int(os.environ.get("KSTAGE", stage))
    import concourse.bass as bass
    import concourse.tile as tile
    from concourse import mybir

    f32 = mybir.dt.float32
    f16 = mybir.dt.float16
    Exp = mybir.ActivationFunctionType.Exp

    from concourse import bacc
    nc = bacc.Bacc(None, target_bir_lowering=False)
    xt_d = nc.dram_tensor("xt", [DIM, N], f16, kind="ExternalInput")
    wqkvt_d = nc.dram_tensor("wqkvt", [DIM, 768], f16, kind="ExternalInput")
    woutt_d = nc.dram_tensor("woutt", [256, DIM], f16, kind="ExternalInput")
    masks_d = nc.dram_tensor("masks", [4, 128, 256], f16, kind="ExternalInput")
    npad_d = nc.dram_tensor("npad", [128, 1], f32, kind="ExternalInput")
    ident_d = nc.dram_tensor("ident", [128, 128], f16, kind="ExternalInput")
    out_d = nc.dram_tensor("out", [N, DIM], f16, kind="ExternalOutput")

    with tile.TileContext(nc) as tc:
        from contextlib import ExitStack
        with ExitStack() as ctx:
            consts = ctx.enter_context(tc.tile_pool(name="consts", bufs=1))

            # wqkvt first: the prologue strips need it immediately; the
            # other consts are DMA'd after the prologue (first used later).
            wqkvt = consts.tile([128, 4, 768], f16)
            wqkvt_v = wqkvt_d.rearrange("(c p) m -> p c m", p=128)
            nc.sync.dma_start(out=wqkvt[:, :, 0:512], in_=wqkvt_v[:, :, 0:512])
            nc.sync.dma_start(out=wqkvt[:, :, 512:768], in_=wqkvt_v[:, :, 512:768])
            woutt = consts.tile([128, 2, DIM], f16)
            masks = consts.tile([128, 4, 256], f16)
            npad = consts.tile([128, 1], f32)
            ident = consts.tile([128, 128], f16)

            def late_const_dmas():
                nc.sync.dma_start(out=woutt,
                                  in_=woutt_d.rearrange("(c p) m -> p c m", p=128))
                nc.sync.dma_start(out=masks,
                                  in_=masks_d.rearrange("c p f -> p c f"))
                nc.sync.dma_start(out=npad, in_=npad_d[:, :])
                nc.sync.dma_start(out=ident, in_=ident_d[:, :])

            # persistent activations: [p, head-pair g, pixel]
            qt = consts.tile([128, 2, N], f16)
            kt = consts.tile([128, 2, NB], f16)
            # v buffer: [p, chunk, 4 heads x (64 + ones col)]
            vsb = consts.tile([128, NCHUNK, 260], f16)

            nc.vector.memset(kt[:, :, 0:128], 0.0)
            nc.vector.memset(kt[:, :, NB - 128:NB], 0.0)
            nc.vector.memset(vsb[:, 0, :], 0.0)
            nc.vector.memset(vsb[:, NCHUNK - 1, :], 0.0)
            # ones columns (after zero memsets of the pad chunks)
            ones_ap = vsb.rearrange("p c (h e) -> p c h e", h=4)[:, :, :, 64:65]
            nc.vector.memset(ones_ap, 1.0)

            # ---------------- fused pipeline ----------------
            with ExitStack() as cctx:
              if stage >= 2:
                  pspw = cctx.enter_context(
                      tc.tile_pool(name="psum_pw", bufs=2, space="PSUM"))
                  psb = cctx.enter_context(
                      tc.tile_pool(name="psum_b", bufs=1, space="PSUM"))
                  pso = cctx.enter_context(
                      tc.tile_pool(name="psum_o", bufs=1, space="PSUM"))
                  psd = cctx.enter_context(
                      tc.tile_pool(name="psum_d", bufs=1, space="PSUM"))
                  # pj (out-projection) and pt (transposes) share one bank:
                  # their lifetimes are staggered within an iteration.
                  pjt = cctx.enter_context(
                      tc.tile_pool(name="psum_pjt", bufs=1, space="PSUM"))
                  xin = cctx.enter_context(tc.tile_pool(name="xin", bufs=2))
                  epool = cctx.enter_context(tc.tile_pool(name="em", bufs=3))
                  dpool = cctx.enter_context(tc.tile_pool(name="den", bufs=2))
                  opool = cctx.enter_context(tc.tile_pool(name="opix", bufs=2))
                  otpool = cctx.enter_context(tc.tile_pool(name="ot", bufs=2))
                  obpool = cctx.enter_context(tc.tile_pool(name="ob", bufs=2))
                  xt_view = xt_d.rearrange("(c p) n -> p c n", p=128)
                  # PE clock warm-up: scratch matmuls on a zeroed tile during
                  # the initial weight/x DMA wait, so the p-state is ramped
                  # when the first real projection chain issues.
                  warm = cctx.enter_context(tc.tile_pool(name="warm", bufs=1))
                  wsb = warm.tile([128, 128], f16)
                  nc.vector.memset(wsb, 0.0)
                  wps = psb.tile([128, 2, 256], f32, tag="ps", name="warmps")
                  for i in range(16):
                      nc.tensor.matmul(wps[:, 0, 0:128], wsb, wsb,
                                       start=True, stop=True)
                  pre_x = {}
                  for t in range(2):
                      xt_pre = xin.tile([128, 4, 256], f16, name=f"xt_pre{t}")
                      nc.sync.dma_start(out=xt_pre,
                                        in_=xt_view[:, :, 256 * t:256 * t + 256])
                      pre_x[t] = xt_pre

                  def strip_proj(t):
                      """q/k/v projection for the 256-px strip [256t, 256t+256).

                      Emitted two iterations ahead of first use; the matmuls
                      are PE filler work behind the critical dots/AV stream.
                      """
                      s0 = 256 * t
                      if t in pre_x:
                          xtile = pre_x.pop(t)
                      else:
                          xtile = xin.tile([128, 4, 256], f16)
                          nc.sync.dma_start(out=xtile,
                                            in_=xt_view[:, :, s0:s0 + 256])
                      for qk in range(2):  # 0: q pair0/1, 1: k pair0/1
                          ps = psb.tile([128, 2, 256], f32, tag="ps", name="psqk")
                          for m in range(2):
                              for kc in range(4):
                                  nc.tensor.matmul(
                                      ps[:, m, :],
                                      wqkvt[:, kc, (2 * qk + m) * 128:
                                            (2 * qk + m + 1) * 128],
                                      xtile[:, kc, :],
                                      start=(kc == 0), stop=(kc == 3))
                          if qk == 0:
                              nc.scalar.copy(qt[:, :, s0:s0 + 256], ps)
                          else:
                              nc.vector.tensor_copy(
                                  kt[:, :, 128 + s0:128 + s0 + 256], ps)
                      psv = psb.tile([128, 2, 256], f32, tag="ps", name="psv")
                      for sub in range(2):
                          for kc in range(4):
                              nc.tensor.matmul(
                                  psv[:, sub, :],
                                  xtile[:, kc, sub * 128:(sub + 1) * 128],
                                  wqkvt[:, kc, 512:768],
                                  start=(kc == 0), stop=(kc == 3))
                      vdst = vsb[:, 1 + 2 * t:3 + 2 * t].rearrange(
                          "p c (h e) -> p c h e", h=4)[:, :, :, 0:64]
                      nc.vector.tensor_copy(
                          vdst, psv.rearrange("p c (h e) -> p c h e", h=4))

                  def dots_exp_mask(si):
                      """QK^T dots + exp + mask for batch si -> em tile."""
                      s = si * 256
                      # em[j', chunk, slot, p']; slot = 2*(h%2) + h//2
                      em = epool.tile([128, 4, 4, 256], f16)
                      if si < 3:
                          # zero the never-exp'd zones once per buffer; the
                          # si loop only ever writes inside CHUNK_WIN after.
                          nc.vector.memset(em[:, 0, :, 130:256], 0.0)
                          nc.vector.memset(em[:, 3, :, 0:126], 0.0)
                      for c in range(4):
                          lo, hi = CHUNK_WIN[c]
                          pw = pspw.tile([128, 2, 2, 256], f32)
                          for g in range(2):
                              for hs in range(2):
                                  lo_p, hi_p = hs * 64, (hs + 1) * 64
                                  nc.tensor.matmul(
                                      pw[:, hs, g, lo:hi],
                                      kt[lo_p:hi_p, g,
                                         s + 128 * c:s + 128 * c + 128],
                                      qt[lo_p:hi_p, g, s + lo:s + hi],
                                      start=True, stop=True)
                          nc.scalar.activation(
                              out=em[:, c, :, lo:hi],
                              in_=pw.rearrange("p a b f -> p (a b) f")[:, :, lo:hi],
                              func=Exp, scale=SCALE)
                          mask_b = masks[:, c, lo:hi].unsqueeze(1).to_broadcast(
                              [128, 4, hi - lo])
                          nc.vector.tensor_mul(em[:, c, :, lo:hi],
                                               em[:, c, :, lo:hi], mask_b)
                      return em

                  def av(si, em):
                      """Attention-weighted V sums + denominators for batch si."""
                      # po[p', ph, gh, e]; pden[p', ph, gh, 0] via ones column
                      po = pso.tile([128, 2, 4, 64], f32)
                      pden = psd.tile([128, 2, 4, 1], f32)
                      for gh in range(4):
                          slot = 2 * (gh % 2) + gh // 2
                          for ph in range(2):
                              plo, phi = 128 * ph, 128 * ph + 128
                              for out_ap, vlo, vn in (
                                      (po, 65 * gh, 64),
                                      (pden, 65 * gh + 64, 1)):
                                  for c in range(4):
                                      nc.tensor.matmul(
                                          out_ap[:, ph, gh, :],
                                          em[:, c, slot, plo:phi],
                                          vsb[:, 2 * si + c, vlo:vlo + vn],
                                          start=(c == 0), stop=(c == 3))
                      return po, pden

                  def post(po, pden):
                      """Normalize + transpose: po/pden -> otb (O^T, fp16)."""
                      den = dpool.tile([128, 2, 4], f32, tag="den")
                      nc.vector.tensor_add(
                          den, pden[:, :, :, 0],
                          npad.unsqueeze(1).to_broadcast([128, 2, 4]))
                      rec = dpool.tile([128, 2, 4], f32, tag="rec")
                      nc.vector.reciprocal(rec, den)
                      opix = opool.tile([128, 2, 4, 64], f16)
                      nc.vector.tensor_mul(
                          opix, po, rec.unsqueeze(3).to_broadcast([128, 2, 4, 64]))
                      # O^T via 4 PE transposes (f16): pt[e-local, ph, i, p]
                      pt = pjt.tile([128, 2, 2, 128], f16, tag="pjt", name="pt")
                      for ph in range(2):
                          for i in range(2):
                              nc.tensor.transpose(
                                  pt[:, ph, i],
                                  opix[:, ph, 2 * i:2 * i + 2, :].rearrange(
                                      "p a e -> p (a e)"),
                                  ident)
                      otb = otpool.tile([128, 2, 2, 128], f16)
                      nc.vector.tensor_copy(otb, pt)
                      return otb

                  def outproj(si, otb):
                      """Partial out-projection + bias for batch si -> DMA out."""
                      s = si * 256
                      ob = obpool.tile([128, 2, DIM], f16)
                      for ph in range(2):
                          pj = pjt.tile([128, DIM], f32, tag="pjt", name="pj")
                          for i in range(2):
                              nc.tensor.matmul(
                                  pj, otb[:, ph, i], woutt[:, i],
                                  start=(i == 0), stop=(i == 1))
                          nc.scalar.copy(ob[:, ph], pj)
                      nc.sync.dma_start(
                          out=out_d[s:s + 256, :].rearrange(
                              "(a p) m -> p a m", a=2, p=128),
                          in_=ob)

                  # Software-pipelined emission: per iteration emit dots(si)
                  # first (lowest priority -> PE prefers them when ready),
                  # then the previous batch's AV, the out-projection two
                  # batches back, the previous batch's normalize/transpose,
                  # and finally the strip projection two batches ahead as PE
                  # filler.  This matches true readiness order so the
                  # scheduler's in-order PE stream never blocks on a
                  # cross-engine chain while newer work is ready.
                  ems, otbs = {}, {}
                  strip_proj(0)
                  strip_proj(1)
                  late_const_dmas()
                  for si in range(18):
                      if si < 16:
                          ems[si] = dots_exp_mask(si)
                      if stage >= 3 and 1 <= si <= 16:
                          po, pden = av(si - 1, ems.pop(si - 1))
                      if stage >= 4 and si >= 2 and (si - 2) in otbs:
                          outproj(si - 2, otbs.pop(si - 2))
                      if stage >= 4 and 1 <= si <= 16:
                          otbs[si - 1] = post(po, pden)
                      if si + 2 <= 15:
                          strip_proj(si + 2)

    nc.finalize()
    return nc


def _prepare_core_inputs(x, w_qkv, w_out, b_out):
    masks, n_pad = _make_masks()
    ident = np.eye(128, dtype=np.float16)
    per_core = []
    for ci in range(N_CORES):
        b, hg = ci // 2, ci % 2
        q_rows = w_qkv[256 * hg:256 * hg + 256]
        k_rows = w_qkv[INNER + 256 * hg:INNER + 256 * hg + 256]
        v_rows = w_qkv[2 * INNER + 256 * hg:2 * INNER + 256 * hg + 256]
        w_slice = np.concatenate([q_rows, k_rows, v_rows], axis=0)  # [768, 512]
        per_core.append({
            "xt": np.ascontiguousarray(x[b].T).astype(np.float16),
            "wqkvt": np.ascontiguousarray(w_slice.T).astype(np.float16),
            "woutt": np.ascontiguousarray(
                w_out[:, 256 * hg:256 * hg + 256].T).astype(np.float16),
            "masks": masks,
            "npad": n_pad,
            "ident": ident,
        })
    return per_core


def kernel(x, w_qkv, w_out, b_out, h, w):
    assert int(h) == HMAP and int(w) == WMAP
    x = np.asarray(x, dtype=np.float32)
    w_qkv = np.asarray(w_qkv, dtype=np.float32)
    w_out = np.asarray(w_out, dtype=np.float32)
    b_out = np.asarray(b_out, dtype=np.float32)

    if "nc" not in _cache:
        _cache["nc"] = _build_nc()
    nc = _cache["nc"]

    from concourse.bass_utils import run_bass_kernel_spmd
    in_maps = _prepare_core_inputs(x, w_qkv, w_out, b_out)
    res = run_bass_kernel_spmd(nc, in_maps, core_ids=list(range(N_CORES)))
    out = np.zeros((B, N, DIM), dtype=np.float32)
    for b in range(B):
        out[b] = (res.results[2 * b]["out"].astype(np.float32)
                  + res.results[2 * b + 1]["out"].astype(np.float32)
                  + b_out)
    return out


# revision 35
# speedup vs baseline: 1.0803x; 1.0547x over previous
"""Trainium2 Bass kernel for nn_LocalAttention (5x5 local window attention).

Contract: kernel(**inputs) takes the FULL inputs from setup_inputs() and
returns the FULL output.  Internally shards across 8 NeuronCores as
(batch b in 0..3) x (head-group hg in 0..1, 4 heads each).  Each core
computes a partial output projection; the host sums the two partials per
batch (and adds the bias half from each core).

Single software-pipelined loop (no separate projection phase): each
iteration emits, in priority order, the QK^T dots/exp/mask for batch si,
the attention-weighted V sums for batch si-1, the out-projection for
batch si-2, the normalize/transpose for batch si-1, and the q/k/v
projection for the 256-px strip si+2 (PE filler work).  This keeps the
PE busy ~77% of the makespan and ramped at full clock.

Per-core algorithm (validated against the reference in numpy):
  - qT,kT (d-major, fp16) and v (pixel-major, fp16, with ones column for
    the softmax denominator) via matmuls from host-pre-transposed
    x.T and w slices, one 256-px strip per iteration.
  - k/v live in buffers padded with 2 zero image-rows top+bottom
    (buffer pixel = image pixel + 128): padded neighbors naturally give
    dots=0 -> exp(0)=1 in the denominator and v=0, matching the
    reference's zero-padded local window.
  - Per 256-pixel batch s: banded transposed pairwise dots
    E_T[j, p] = k_buf[s+j] . q[s+p] for j in [0,512) as 4 chunks of 128,
    restricted to the valid column window per chunk ([0,130) for c=0,
    [126,256) for c=3).
  - exp on ACT (scale=1/8) over the valid column slices only; in-place
    multiply by a precomputed window/wrap mask on DVE (fp16 2x mode;
    GPSIMD cannot read PSUM and is 2.6x slower per element, so it gets
    no regular work); column-wrapped neighbors are masked out and
    re-added to the denominator via n_pad.
  - Weighted sum over v: one accumulating matmul chain per (head,
    pixel-half); out-of-window em columns are zeroed once per em buffer
    so full-width stationaries are safe.  A parallel 1-column chain
    with the ones column accumulates the denominator (pden).
  - den = pden + n_pad, reciprocal on DVE, single fused normalize
    multiply (PSUM po * broadcast recip -> fp16 opix).
  - O^T via 4 fp16 PE transposes into one PSUM bank shared with the
    out-projection accumulator (staggered lifetimes); psum->sbuf copies
    split between ACT and DVE; b_out is added on the host during the
    partial-sum gather; fp16 output DMA.
"""

import numpy as np

B, HMAP, WMAP = 4, 64, 64
N = HMAP * WMAP          # 4096
DIM = 512
HEADS, HEAD_DIM = 8, 64
INNER = HEADS * HEAD_DIM  # 512
SCALE = HEAD_DIM ** -0.5
NB = N + 256             # padded k/v buffer pixels (2 zero rows each side)
NCHUNK = NB // 128       # 34
N_CORES = 8

_cache = {}


def _make_masks():
    """Window/wrap masks for the 4 chunks of a 256-px batch, plus n_pad.

    mask[c, j', p'] = 1 iff o = 128*c + j' - p' - 128 decomposes as
    64*di + dj with |di|,|dj| <= 2 and column p'%64 + dj stays in-image.
    n_pad[p] = number of column-invalid window positions for column p%64.
    """
    o = (128 * np.arange(4)[:, None, None] + np.arange(128)[None, :, None]
         - np.arange(256)[None, None, :] - 128)           # [4,128,256]
    di = np.round(o / 64.0).astype(np.int64)
    dj = o - 64 * di
    col = (np.arange(256) % 64)[None, None, :]
    ok = (np.abs(di) <= 2) & (np.abs(dj) <= 2) & (col + dj >= 0) & (col + dj < 64)
    masks = ok.astype(np.float16)
    colv = np.arange(64)
    npad_col = np.zeros(64, dtype=np.float32)
    for djv in range(-2, 3):
        npad_col += 5.0 * ((colv + djv < 0) | (colv + djv >= 64))
    n_pad = np.tile(npad_col, 2).reshape(128, 1).astype(np.float32)
    return masks, n_pad


# valid q-column window [lo, hi) of each 128-row k-chunk of a 256-px batch
CHUNK_WIN = [(0, 130), (0, 256), (0, 256), (126, 256)]


def _build_nc(stage=99):
    import os
    stage = int(os.environ.get("KSTAGE", stage))
    import concourse.bass as bass
    import concourse.tile as tile
    from concourse import mybir

    f32 = mybir.dt.float32
    f16 = mybir.dt.float16
    Exp = mybir.ActivationFunctionType.Exp

    from concourse import bacc
    nc = bacc.Bacc(None, target_bir_lowering=False)
    xt_d = nc.dram_tensor("xt", [DIM, N], f16, kind="ExternalInput")
    wqkvt_d = nc.dram_tensor("wqkvt", [DIM, 768], f16, kind="ExternalInput")
    woutt_d = nc.dram_tensor("woutt", [256, DIM], f16, kind="ExternalInput")
    masks_d = nc.dram_tensor("masks", [4, 128, 256], f16, kind="ExternalInput")
    npad_d = nc.dram_tensor("npad", [128, 1], f32, kind="ExternalInput")
    ident_d = nc.dram_tensor("ident", [128, 128], f16, kind="ExternalInput")
    out_d = nc.dram_tensor("out", [N, DIM], f16, kind="ExternalOutput")

    with tile.TileContext(nc) as tc:
        from contextlib import ExitStack
        with ExitStack() as ctx:
            consts = ctx.enter_context(tc.tile_pool(name="consts", bufs=1))

            # wqkvt first: the prologue strips need it immediately; the
            # other consts are DMA'd after the prologue (first used later).
            wqkvt = consts.tile([128, 4, 768], f16)
            wqkvt_v = wqkvt_d.rearrange("(c p) m -> p c m", p=128)
            for blk0 in range(3):
                nc.sync.dma_start(
                    out=wqkvt[:, :, 256 * blk0:256 * blk0 + 256],
                    in_=wqkvt_v[:, :, 256 * blk0:256 * blk0 + 256])
            woutt = consts.tile([128, 2, DIM], f16)
            masks = consts.tile([128, 4, 256], f16)
            npad = consts.tile([128, 1], f32)
            ident = consts.tile([128, 128], f16)

            def late_const_dmas():
                nc.sync.dma_start(out=woutt,
                                  in_=woutt_d.rearrange("(c p) m -> p c m", p=128))
                nc.sync.dma_start(out=masks,
                                  in_=masks_d.rearrange("c p f -> p c f"))
                nc.sync.dma_start(out=npad, in_=npad_d[:, :])
                nc.sync.dma_start(out=ident, in_=ident_d[:, :])

            # persistent activations: [p, head-pair g, pixel]
            qt = consts.tile([128, 2, N], f16)
            kt = consts.tile([128, 2, NB], f16)
            # v buffer: [p, chunk, 4 heads x (64 + ones col)]
            vsb = consts.tile([128, NCHUNK, 260], f16)

            nc.vector.memset(kt[:, :, 0:128], 0.0)
            nc.vector.memset(kt[:, :, NB - 128:NB], 0.0)
            nc.vector.memset(vsb[:, 0, :], 0.0)
            nc.vector.memset(vsb[:, NCHUNK - 1, :], 0.0)
            # ones columns (after zero memsets of the pad chunks)
            ones_ap = vsb.rearrange("p c (h e) -> p c h e", h=4)[:, :, :, 64:65]
            nc.vector.memset(ones_ap, 1.0)

            # ---------------- fused pipeline ----------------
            with ExitStack() as cctx:
              if stage >= 2:
                  pspw = cctx.enter_context(
                      tc.tile_pool(name="psum_pw", bufs=2, space="PSUM"))
                  psb = cctx.enter_context(
                      tc.tile_pool(name="psum_b", bufs=1, space="PSUM"))
                  pso = cctx.enter_context(
                      tc.tile_pool(name="psum_o", bufs=1, space="PSUM"))
                  psd = cctx.enter_context(
                      tc.tile_pool(name="psum_d", bufs=1, space="PSUM"))
                  # pj (out-projection) and pt (transposes) share one bank:
                  # their lifetimes are staggered within an iteration.
                  pjt = cctx.enter_context(
                      tc.tile_pool(name="psum_pjt", bufs=1, space="PSUM"))
                  xin = cctx.enter_context(tc.tile_pool(name="xin", bufs=2))
                  epool = cctx.enter_context(tc.tile_pool(name="em", bufs=3))
                  dpool = cctx.enter_context(tc.tile_pool(name="den", bufs=2))
                  opool = cctx.enter_context(tc.tile_pool(name="opix", bufs=2))
                  otpool = cctx.enter_context(tc.tile_pool(name="ot", bufs=2))
                  obpool = cctx.enter_context(tc.tile_pool(name="ob", bufs=2))
                  xt_view = xt_d.rearrange("(c p) n -> p c n", p=128)
                  pool_tags = {id(psb): "ps", id(pso): "po", id(psd): "pden",
                               id(pjt): "pjt"}
                  # PE clock warm-up: scratch matmuls on a zeroed tile during
                  # the initial weight/x DMA wait, so the p-state is ramped
                  # when the first real projection chain issues.
                  warm = cctx.enter_context(tc.tile_pool(name="warm", bufs=1))
                  wsb = warm.tile([128, 128], f16)
                  nc.vector.memset(wsb, 0.0)
                  wps = psb.tile([128, 2, 256], f32, tag="ps", name="warmps")
                  for i in range(16):
                      nc.tensor.matmul(wps[:, 0, 0:128], wsb, wsb,
                                       start=True, stop=True)
                  pre_x = {}
                  for t in range(2):
                      xt_pre = xin.tile([128, 4, 256], f16, name=f"xt_pre{t}")
                      nc.sync.dma_start(out=xt_pre,
                                        in_=xt_view[:, :, 256 * t:256 * t + 256])
                      pre_x[t] = xt_pre

                  def strip_proj(t, pools=None):
                      """q/k/v projection for the 256-px strip [256t, 256t+256).

                      Emitted two iterations ahead of first use; the matmuls
                      are PE filler work behind the critical dots/AV stream.
                      The prologue strips spread their chains across the
                      not-yet-used psum pools so they don't serialize through
                      psb's single bank.
                      """
                      s0 = 256 * t
                      if t in pre_x:
                          xtile = pre_x.pop(t)
                      else:
                          xtile = xin.tile([128, 4, 256], f16)
                          nc.sync.dma_start(out=xtile,
                                            in_=xt_view[:, :, s0:s0 + 256])
                      if pools is None:
                          pools = [psb, psb, psb]
                      for qk in range(2):  # 0: q pair0/1, 1: k pair0/1
                          pool_k = pools[qk]
                          ps = pool_k.tile([128, 2, 256], f32,
                                           tag=pool_tags[id(pool_k)],
                                           name="psqk_pro")
                          for m in range(2):
                              for kc in range(4):
                                  nc.tensor.matmul(
                                      ps[:, m, :],
                                      wqkvt[:, kc, (2 * qk + m) * 128:
                                            (2 * qk + m + 1) * 128],
                                      xtile[:, kc, :],
                                      start=(kc == 0), stop=(kc == 3))
                          if qk == 0:
                              nc.scalar.copy(qt[:, :, s0:s0 + 256], ps)
                          else:
                              nc.vector.tensor_copy(
                                  kt[:, :, 128 + s0:128 + s0 + 256], ps)
                      pool_v = pools[2]
                      psv = pool_v.tile([128, 2, 256], f32,
                                        tag=pool_tags[id(pool_v)],
                                        name="psv_pro")
                      for sub in range(2):
                          for kc in range(4):
                              nc.tensor.matmul(
                                  psv[:, sub, :],
                                  xtile[:, kc, sub * 128:(sub + 1) * 128],
                                  wqkvt[:, kc, 512:768],
                                  start=(kc == 0), stop=(kc == 3))
                      vdst = vsb[:, 1 + 2 * t:3 + 2 * t].rearrange(
                          "p c (h e) -> p c h e", h=4)[:, :, :, 0:64]
                      nc.vector.tensor_copy(
                          vdst, psv.rearrange("p c (h e) -> p c h e", h=4))

                  def dots_exp_mask(si):
                      """QK^T dots + exp + mask for batch si -> em tile."""
                      s = si * 256
                      # em[j', chunk, slot, p']; slot = 2*(h%2) + h//2
                      em = epool.tile([128, 4, 4, 256], f16)
                      if si < 3:
                          # zero the never-exp'd zones once per buffer; the
                          # si loop only ever writes inside CHUNK_WIN after.
                          nc.vector.memset(em[:, 0, :, 130:256], 0.0)
                          nc.vector.memset(em[:, 3, :, 0:126], 0.0)
                      for c in range(4):
                          lo, hi = CHUNK_WIN[c]
                          pw = pspw.tile([128, 2, 2, 256], f32)
                          for g in range(2):
                              for hs in range(2):
                                  lo_p, hi_p = hs * 64, (hs + 1) * 64
                                  nc.tensor.matmul(
                                      pw[:, hs, g, lo:hi],
                                      kt[lo_p:hi_p, g,
                                         s + 128 * c:s + 128 * c + 128],
                                      qt[lo_p:hi_p, g, s + lo:s + hi],
                                      start=True, stop=True)
                          nc.scalar.activation(
                              out=em[:, c, :, lo:hi],
                              in_=pw.rearrange("p a b f -> p (a b) f")[:, :, lo:hi],
                              func=Exp, scale=SCALE)
                          mask_b = masks[:, c, lo:hi].unsqueeze(1).to_broadcast(
                              [128, 4, hi - lo])
                          nc.vector.tensor_mul(em[:, c, :, lo:hi],
                                               em[:, c, :, lo:hi], mask_b)
                      return em

                  def av(si, em):
                      """Attention-weighted V sums + denominators for batch si."""
                      # po[p', ph, gh, e]; pden[p', ph, gh, 0] via ones column
                      po = pso.tile([128, 2, 4, 64], f32, tag="po")
                      pden = psd.tile([128, 2, 4, 1], f32, tag="pden")
                      for gh in range(4):
                          slot = 2 * (gh % 2) + gh // 2
                          for ph in range(2):
                              plo, phi = 128 * ph, 128 * ph + 128
                              for out_ap, vlo, vn in (
                                      (po, 65 * gh, 64),
                                      (pden, 65 * gh + 64, 1)):
                                  for c in range(4):
                                      nc.tensor.matmul(
                                          out_ap[:, ph, gh, :],
                                          em[:, c, slot, plo:phi],
                                          vsb[:, 2 * si + c, vlo:vlo + vn],
                                          start=(c == 0), stop=(c == 3))
                      return po, pden

                  def post(po, pden):
                      """Normalize + transpose: po/pden -> otb (O^T, fp16)."""
                      den = dpool.tile([128, 2, 4], f32, tag="den")
                      nc.vector.tensor_add(
                          den, pden[:, :, :, 0],
                          npad.unsqueeze(1).to_broadcast([128, 2, 4]))
                      rec = dpool.tile([128, 2, 4], f32, tag="rec")
                      nc.vector.reciprocal(rec, den)
                      opix = opool.tile([128, 2, 4, 64], f16)
                      nc.vector.tensor_mul(
                          opix, po, rec.unsqueeze(3).to_broadcast([128, 2, 4, 64]))
                      # O^T via 4 PE transposes (f16): pt[e-local, ph, i, p]
                      pt = pjt.tile([128, 2, 2, 128], f16, tag="pjt", name="pt")
                      for ph in range(2):
                          for i in range(2):
                              nc.tensor.transpose(
                                  pt[:, ph, i],
                                  opix[:, ph, 2 * i:2 * i + 2, :].rearrange(
                                      "p a e -> p (a e)"),
                                  ident)
                      otb = otpool.tile([128, 2, 2, 128], f16)
                      nc.vector.tensor_copy(otb, pt)
                      return otb

                  def outproj(si, otb):
                      """Partial out-projection + bias for batch si -> DMA out."""
                      s = si * 256
                      ob = obpool.tile([128, 2, DIM], f16)
                      for ph in range(2):
                          pj = pjt.tile([128, DIM], f32, tag="pjt", name="pj")
                          for i in range(2):
                              nc.tensor.matmul(
                                  pj, otb[:, ph, i], woutt[:, i],
                                  start=(i == 0), stop=(i == 1))
                          nc.scalar.copy(ob[:, ph], pj)
                      nc.sync.dma_start(
                          out=out_d[s:s + 256, :].rearrange(
                              "(a p) m -> p a m", a=2, p=128),
                          in_=ob)

                  # Software-pipelined emission: per iteration emit dots(si)
                  # first (lowest priority -> PE prefers them when ready),
                  # then the previous batch's AV, the out-projection two
                  # batches back, the previous batch's normalize/transpose,
                  # and finally the strip projection two batches ahead as PE
                  # filler.  This matches true readiness order so the
                  # scheduler's in-order PE stream never blocks on a
                  # cross-engine chain while newer work is ready.
                  ems, otbs = {}, {}
                  strip_proj(0, pools=[psb, pso, psd])
                  strip_proj(1, pools=[pjt, pso, psb])
                  late_const_dmas()
                  for si in range(18):
                      if si < 16:
                          ems[si] = dots_exp_mask(si)
                      if stage >= 4 and si >= 2 and (si - 2) in otbs:
                          outproj(si - 2, otbs.pop(si - 2))
                      if stage >= 3 and 1 <= si <= 16:
                          po, pden = av(si - 1, ems.pop(si - 1))
                      if si + 2 <= 15:
                          strip_proj(si + 2)
                      if stage >= 4 and 1 <= si <= 16:
                          otbs[si - 1] = post(po, pden)

    nc.finalize()
    return nc


def _prepare_core_inputs(x, w_qkv, w_out, b_out):
    masks, n_pad = _make_masks()
    ident = np.eye(128, dtype=np.float16)
    per_core = []
    for ci in range(N_CORES):
        b, hg = ci // 2, ci % 2
        q_rows = w_qkv[256 * hg:256 * hg + 256]
        k_rows = w_qkv[INNER + 256 * hg:INNER + 256 * hg + 256]
        v_rows = w_qkv[2 * INNER + 256 * hg:2 * INNER + 256 * hg + 256]
        w_slice = np.concatenate([q_rows, k_rows, v_rows], axis=0)  # [768, 512]
        per_core.append({
            "xt": np.ascontiguousarray(x[b].T).astype(np.float16),
            "wqkvt": np.ascontiguousarray(w_slice.T).astype(np.float16),
            "woutt": np.ascontiguousarray(
                w_out[:, 256 * hg:256 * hg + 256].T).astype(np.float16),
            "masks": masks,
            "npad": n_pad,
            "ident": ident,
        })
    return per_core


def kernel(x, w_qkv, w_out, b_out, h, w):
    assert int(h) == HMAP and int(w) == WMAP
    x = np.asarray(x, dtype=np.float32)
    w_qkv = np.asarray(w_qkv, dtype=np.float32)
    w_out = np.asarray(w_out, dtype=np.float32)
    b_out = np.asarray(b_out, dtype=np.float32)

    if "nc" not in _cache:
        _cache["nc"] = _build_nc()
    nc = _cache["nc"]

    from concourse.bass_utils import run_bass_kernel_spmd
    in_maps = _prepare_core_inputs(x, w_qkv, w_out, b_out)
    res = run_bass_kernel_spmd(nc, in_maps, core_ids=list(range(N_CORES)))
    out = np.zeros((B, N, DIM), dtype=np.float32)
    for b in range(B):
        out[b] = (res.results[2 * b]["out"].astype(np.float32)
                  + res.results[2 * b + 1]["out"].astype(np.float32)
                  + b_out)
    return out
